# revision 1
# baseline (speedup 1.0000x reference)
"""DTNN layer kernel for Trainium2 (8 NeuronCores).

Math: out[b,i,o] = sum_j sum_h Wfc[o,h] * hx[b,i,h] * hd[b,i,j,h]
with hx = x@Wcf.T + bcf, hd = dist@Wdf.T + bdf.
Since Wfc/Wdf are linear, the j-sum commutes:
    ds[b,i,d]  = sum_j dist[b,i,j,d]                  (memory-bound reduction)
    out[b,i,:] = ((x@Wcf.T + bcf) * (ds@Wdf.T + N*bdf)) @ Wfc.T
So the kernel streams `distance` once (134MB) and does a few 128x128 matmuls.

Sharding: flatten (B,N) -> 1024 i-rows, 128 rows per core; no cross-core comms.

Measured (NTFF profile, core 0): ~70us/core, vs ~47us pure HBM stream at the
358 GB/s per-core fair share plus ~13us fixed NEFF prologue/epilogue and a
~9us serial tail. Structure:
- dist is streamed as a few big HWDGE DMAs on one ring (in-order arrivals);
  DVE folds each tile to 128 columns in place right after it lands (halving
  unit-stride adds run at full DVE rate; strided reduces were 1.6x slower).
- biases are folded into PE matmuls as K=1 rank-1 updates, and the
  (hx * N*bdf) @ WfcT bias term is preloaded into the output PSUM during the
  stream so the post-stream tail is just transpose -> Wdf matmul -> mul ->
  accumulate-matmul -> store.
"""

import numpy as np

import concourse.bass as bass
import concourse.bacc as bacc
import concourse.mybir as mybir
from concourse.tile import TileContext
from concourse.bass_utils import run_bass_kernel_spmd

B, N, D, H = 4, 256, 128, 128
NCORES = 8
ROWS = B * N // NCORES  # 128 i-rows per core
FP = mybir.dt.float32

# packed constant columns: [xT | wcfT | wdfT | wfcT | eye | rows...]
C_XT = 0
C_WCF = 128
C_WDF = 256
C_WFC = 384
C_EYE = 512
C_BCFR = 640   # partition 0: bcf row (1, H)
C_BDFR = 768   # partition 0: bdf row (1, H)
C_ONES = 896   # partition 0: ones row (1, ROWS)
C_BDFC = 1024  # bdf as a per-partition column (H, 1)
C_TOT = 1025


def build_nc():
    nc = bacc.Bacc("TRN2", target_bir_lowering=False)
    dist = nc.declare_dram_parameter("dist", [ROWS, N * D], FP, isOutput=False)
    cst = nc.declare_dram_parameter("cst", [128, C_TOT], FP, isOutput=False)
    out = nc.declare_dram_parameter("out", [ROWS, D], FP, isOutput=True)

    with TileContext(nc) as tc:
        with (
            tc.tile_pool(name="const", bufs=1) as cpool,
            tc.tile_pool(name="dist", bufs=1) as dpool,
            tc.tile_pool(name="work", bufs=1) as wpool,
            tc.tile_pool(name="psum", bufs=1, space="PSUM") as ppool,
        ):
            # Issue the dist stream first so the big DMAs start ASAP; the
            # constants ride behind them on the same queue.
            SIZES = [64, 64, 64, 32, 16, 8, 4, 4]  # j-counts per DMA tile
            dtiles = []
            off = 0
            for k, jn in enumerate(SIZES):
                t = dpool.tile([ROWS, jn * D], FP, tag=f"dist{k}")
                # Single HWDGE ring (SP): in-order arrivals matching the DVE
                # fold order; the stream is HBM-fair-share-bound (~358GB/s)
                # so a second ring adds no bandwidth, only ordering jitter.
                nc.sync.dma_start(out=t[:], in_=dist[:, off * D:(off + jn) * D])
                dtiles.append(t)
                off += jn

            cst_t = cpool.tile([128, C_TOT], FP)
            nc.scalar.dma_start(out=cst_t[:], in_=cst[:])
            xT_t = cst_t[:, C_XT:C_XT + ROWS]
            wcf_t = cst_t[:, C_WCF:C_WCF + H]
            wdf_t = cst_t[:, C_WDF:C_WDF + H]
            wfc_t = cst_t[:, C_WFC:C_WFC + D]
            ident = cst_t[:, C_EYE:C_EYE + ROWS]
            bcf_row = cst_t[0:1, C_BCFR:C_BCFR + H]
            ones_row = cst_t[0:1, C_ONES:C_ONES + ROWS]

            # hx^T = (Wcf^T)^T @ x^T + bcf x ones -> (H, ROWS) in PSUM
            hx_ps = ppool.tile([H, ROWS], FP)
            nc.tensor.matmul(hx_ps[:], wcf_t, xT_t, start=True, stop=False)
            nc.tensor.matmul(hx_ps[:], bcf_row, ones_row, start=False, stop=True)
            hxT = wpool.tile([H, ROWS], FP)
            nc.vector.tensor_copy(hxT[:], hx_ps[:])

            # Preload the bias term (hx * N*bdf) @ Wfc^T into the output
            # PSUM during the stream; the tail's out-matmul accumulates
            # onto it, removing the bias matmul from the critical tail.
            bdfN = wpool.tile([H, 1], FP)
            nc.vector.tensor_scalar_mul(bdfN[:], cst_t[:, C_BDFC:C_BDFC + 1],
                                        float(N))
            s0T = wpool.tile([H, ROWS], FP)
            nc.vector.tensor_scalar_mul(s0T[:], hxT[:], bdfN[:])
            out_ps = ppool.tile([ROWS, D], FP)
            nc.tensor.matmul(out_ps[:], s0T[:], wfc_t, start=True, stop=False)

            # Streaming j-reduction: ds[i,d] = sum_j dist[i,j,d].
            # Each tile is folded to 128 columns in place immediately after
            # its DMA lands (halving adds, all unit-stride = full DVE rate),
            # then added into the running accumulator (tile 0). Per-tile DVE
            # work (~4.9us) keeps pace with per-tile DMA arrival (~5.1us),
            # so only ~2us of DVE work remains after the last (half-size)
            # tile arrives.
            acc = dtiles[0]
            for k, jn in enumerate(SIZES):
                t = dtiles[k]
                half = jn * D // 2
                while half >= D:
                    nc.vector.tensor_add(
                        t[:, 0:half], t[:, 0:half], t[:, half:2 * half]
                    )
                    half //= 2
                if k > 0:
                    nc.vector.tensor_add(acc[:, 0:D], acc[:, 0:D], t[:, 0:D])
            ds = acc[:, 0:D]

            # ds (i,d) -> dsT (d,i) via PE transpose
            dsT_ps = ppool.tile([D, ROWS], FP)
            nc.tensor.transpose(dsT_ps[:], ds, ident)
            dsT = wpool.tile([D, ROWS], FP)
            nc.vector.tensor_copy(dsT[:], dsT_ps[:])

            # hd^T (bias-free) = (Wdf^T)^T @ ds^T -> (H, ROWS)
            hd_ps = ppool.tile([H, ROWS], FP)
            nc.tensor.matmul(hd_ps[:], wdf_t, dsT[:], start=True, stop=True)

            # s^T = hx^T * hd^T (one PSUM operand max per DVE op)
            sT = wpool.tile([H, ROWS], FP)
            nc.vector.tensor_mul(sT[:], hd_ps[:], hxT[:])

            # out += sT^T @ Wfc^T, accumulating onto the preloaded bias term
            nc.tensor.matmul(out_ps[:], sT[:], wfc_t, start=False, stop=True,
                             skip_group_check=True)
            out_sb = wpool.tile([ROWS, D], FP)
            nc.vector.tensor_copy(out_sb[:], out_ps[:])
            nc.sync.dma_start(out=out[:], in_=out_sb[:])
    nc.compile()
    return nc


_NC_CACHE = None


def _get_nc():
    global _NC_CACHE
    if _NC_CACHE is None:
        _NC_CACHE = build_nc()
    return _NC_CACHE


def _make_in_maps(x, distance, Wcf_w, Wcf_b, Wdf_w, Wdf_b, Wfc_w):
    x = np.ascontiguousarray(np.asarray(x, np.float32))
    distance = np.ascontiguousarray(np.asarray(distance, np.float32))
    x_flat = x.reshape(B * N, D)
    dist_flat = distance.reshape(B * N, N * D)
    wcfT = np.asarray(Wcf_w, np.float32).T
    wdfT = np.asarray(Wdf_w, np.float32).T
    wfcT = np.asarray(Wfc_w, np.float32).T
    bcf = np.asarray(Wcf_b, np.float32)
    bdf = np.asarray(Wdf_b, np.float32)
    in_maps = []
    for c in range(NCORES):
        sl = slice(c * ROWS, (c + 1) * ROWS)
        cstblk = np.zeros((128, C_TOT), np.float32)
        cstblk[:, C_XT:C_XT + ROWS] = x_flat[sl].T
        cstblk[:, C_WCF:C_WCF + H] = wcfT
        cstblk[:, C_WDF:C_WDF + H] = wdfT
        cstblk[:, C_WFC:C_WFC + D] = wfcT
        cstblk[:, C_EYE:C_EYE + ROWS] = np.eye(ROWS, dtype=np.float32)
        cstblk[0, C_BCFR:C_BCFR + H] = bcf
        cstblk[0, C_BDFR:C_BDFR + H] = bdf
        cstblk[0, C_ONES:C_ONES + ROWS] = 1.0
        cstblk[:, C_BDFC] = bdf
        in_maps.append({
            "dist": np.ascontiguousarray(dist_flat[sl]),
            "cst": cstblk,
        })
    return in_maps


def kernel(x, distance, Wcf_w, Wcf_b, Wdf_w, Wdf_b, Wfc_w):
    in_maps = _make_in_maps(x, distance, Wcf_w, Wcf_b, Wdf_w, Wdf_b, Wfc_w)
    nc = _get_nc()
    res = run_bass_kernel_spmd(nc, in_maps, list(range(NCORES))).results
    out = np.concatenate([res[c]["out"] for c in range(NCORES)], axis=0)
    return out.reshape(B, N, D)



# revision 3
# speedup vs baseline: 1.0824x; 1.0824x over previous
"""DTNN layer kernel for Trainium2 (8 NeuronCores).

Math: out[b,i,o] = sum_j sum_h Wfc[o,h] * hx[b,i,h] * hd[b,i,j,h]
with hx = x@Wcf.T + bcf, hd = dist@Wdf.T + bdf.
Since Wfc/Wdf are linear, the j-sum commutes:
    ds[b,i,d]  = sum_j dist[b,i,j,d]                  (memory-bound reduction)
    out[b,i,:] = ((x@Wcf.T + bcf) * (ds@Wdf.T + N*bdf)) @ Wfc.T
So the kernel streams `distance` once (134MB) and does a few 128x128 matmuls.

Sharding: flatten (B,N) -> 1024 i-rows, 128 rows per core; no cross-core comms.

Schedule (from NTFF trace analysis of the previous ~70us version):
- Constants ride FIRST on the same sync-queue DMA stream as dist (230KB,
  ~0.6us) instead of the scalar queue, whose small packets serialized on one
  DMA engine and blocked every DVE op (in-order engine!) until t=29us.
- dist streams as 14 tapered tiles [32x6,16x2,8x3,4,2,2] j-columns; each is
  folded to 128 columns by halving adds as soon as it lands.  DVE f32 adds
  run at ~1 elem/cycle; total fold ~38us vs ~45us of stream, so the fold
  tracks the stream.  Three mid/late tiles fold on GpSimd (Pool) instead
  (f32 adds at ~0.5 elem/ns) so DVE enters the stream tail caught up.
- Each tile's last halving add writes a bf16 partial; PE transpose-
  accumulates the partials into a PSUM dsT bank via bf16 matmuls with a
  bf16 identity (single-pass, ~0.5us, vs 1.6us for fp32 LOW_HIGH).  This
  removes all accumulator adds and the final transpose from the tail.
- Tail after the last byte: last fold -> transpose-accum(stop) -> copy
  dsT(bf16) -> hd matmul(bf16) -> sT mul -> out matmul(bf16, accumulates
  onto a PSUM preloaded during the stream with the (hx*N*bdf)@WfcT bias
  term) -> copy -> DMA.  bf16 end-to-end rel err ~2e-3 (gate is 2e-2).
"""

import numpy as np
from ml_dtypes import bfloat16

import concourse.bass as bass
import concourse.bacc as bacc
import concourse.mybir as mybir
from concourse.tile import TileContext
from concourse.bass_utils import run_bass_kernel_spmd

B, N, D, H = 4, 256, 128, 128
NCORES = 8
ROWS = B * N // NCORES  # 128 i-rows per core
FP = mybir.dt.float32
BF = mybir.dt.bfloat16

# dist tile taper (j-columns per DMA) and which tiles fold on GpSimd
SIZES = [32, 32, 32, 32, 32, 32, 16, 16, 8, 8, 8, 4, 2, 2]
POOL_TILES = {6, 8, 11}
assert sum(SIZES) == N

# f32 constant block columns: [xT | wcfT | bcf_col | bdf_col]
CF_XT = 0
CF_WCF = 128
CF_BCF = 256
CF_BDF = 257
CF_TOT = 258
# bf16 constant block columns: [wdfT | wfcT | eye]
CB_WDF = 0
CB_WFC = 128
CB_EYE = 256
CB_TOT = 384


def build_nc():
    nc = bacc.Bacc("TRN2", target_bir_lowering=False)
    dist = nc.declare_dram_parameter("dist", [ROWS, N * D], FP, isOutput=False)
    cstf = nc.declare_dram_parameter("cstf", [128, CF_TOT], FP, isOutput=False)
    cstb = nc.declare_dram_parameter("cstb", [128, CB_TOT], BF, isOutput=False)
    out = nc.declare_dram_parameter("out", [ROWS, D], FP, isOutput=True)

    with TileContext(nc) as tc:
        with (
            tc.tile_pool(name="const", bufs=1) as cpool,
            tc.tile_pool(name="dist", bufs=1) as dpool,
            tc.tile_pool(name="work", bufs=1) as wpool,
            tc.tile_pool(name="psum", bufs=1, space="PSUM") as ppool,
        ):
            # Constants first on the sync queue: they land ~0.6us before the
            # first dist tile, unblocking the in-order DVE/PE streams early.
            cstf_t = cpool.tile([128, CF_TOT], FP, tag="cstf")
            nc.sync.dma_start(out=cstf_t[:], in_=cstf[:])
            cstb_t = cpool.tile([128, CB_TOT], BF, tag="cstb")
            nc.sync.dma_start(out=cstb_t[:], in_=cstb[:])

            dtiles = []
            off = 0
            for k, jn in enumerate(SIZES):
                t = dpool.tile([ROWS, jn * D], FP, tag=f"dist{k}")
                nc.sync.dma_start(out=t[:], in_=dist[:, off * D:(off + jn) * D])
                dtiles.append(t)
                off += jn

            xT_t = cstf_t[:, CF_XT:CF_XT + ROWS]
            wcf_t = cstf_t[:, CF_WCF:CF_WCF + H]
            bcf_col = cstf_t[:, CF_BCF:CF_BCF + 1]
            bdf_col = cstf_t[:, CF_BDF:CF_BDF + 1]
            wdf_b = cstb_t[:, CB_WDF:CB_WDF + H]
            wfc_b = cstb_t[:, CB_WFC:CB_WFC + D]
            eye_b = cstb_t[:, CB_EYE:CB_EYE + ROWS]

            # hx^T = (Wcf^T)^T @ x^T -> (H, ROWS) in PSUM (fp32)
            hx_ps = ppool.tile([H, ROWS], FP, tag="hx")
            nc.tensor.matmul(hx_ps[:], wcf_t, xT_t, start=True, stop=True)

            # bf16 fold partials, one per tile
            folds = [wpool.tile([ROWS, D], BF, tag=f"fold{k}",
                                name=f"fold{k}")
                     for k in range(len(SIZES))]

            # DVE program order: fold0 first (gated only on dist tile 0),
            # then the two cst-dependent aux ops, then the rest of the folds.
            def emit_fold(k):
                eng = nc.gpsimd if k in POOL_TILES else nc.vector
                t = dtiles[k]
                jn = SIZES[k]
                half = jn * D // 2
                while half > D:
                    eng.tensor_add(t[:, 0:half], t[:, 0:half],
                                   t[:, half:2 * half])
                    half //= 2
                # last halving add writes the bf16 partial
                eng.tensor_add(folds[k][:], t[:, 0:D], t[:, D:2 * D])

            emit_fold(0)

            # hx^T + bcf (f32), s0T = hxT * bdf * N (bf16, cast on write)
            hxT = wpool.tile([H, ROWS], FP, tag="hxT")
            nc.vector.tensor_scalar(hxT[:], hx_ps[:], bcf_col, None,
                                    mybir.AluOpType.add)
            s0T = wpool.tile([H, ROWS], BF, tag="s0T")
            nc.vector.tensor_scalar(s0T[:], hxT[:], bdf_col, float(N),
                                    mybir.AluOpType.mult,
                                    mybir.AluOpType.mult)

            for k in range(1, len(SIZES)):
                emit_fold(k)

            # Preload the bias term (hx * N*bdf) @ Wfc^T into the output
            # PSUM during the stream (bf16 matmul).
            out_ps = ppool.tile([ROWS, D], FP, tag="out")
            nc.tensor.matmul(out_ps[:], s0T[:], wfc_b, start=True, stop=False,
                             skip_group_check=True)

            # PE transpose-accumulate each bf16 partial into dsT PSUM:
            # dsT_ps += fold_k^T @ eye
            dsT_ps = ppool.tile([D, ROWS], FP, tag="dsT")
            for k in range(len(SIZES)):
                nc.tensor.matmul(dsT_ps[:], folds[k][:], eye_b,
                                 start=(k == 0), stop=(k == len(SIZES) - 1),
                                 skip_group_check=True)

            dsT = wpool.tile([D, ROWS], BF, tag="dsTb")
            nc.vector.tensor_copy(dsT[:], dsT_ps[:])

            # hd^T (bias-free) = (Wdf^T)^T @ ds^T -> (H, ROWS), bf16 matmul
            hd_ps = ppool.tile([H, ROWS], FP, tag="hd")
            nc.tensor.matmul(hd_ps[:], wdf_b, dsT[:], start=True, stop=True,
                             skip_group_check=True)

            # s^T = hx^T * hd^T (bf16 out, cast on write)
            sT = wpool.tile([H, ROWS], BF, tag="sT")
            nc.vector.tensor_mul(sT[:], hd_ps[:], hxT[:])

            # out += sT^T @ Wfc^T, accumulating onto the preloaded bias term
            nc.tensor.matmul(out_ps[:], sT[:], wfc_b, start=False, stop=True,
                             skip_group_check=True)
            out_sb = wpool.tile([ROWS, D], FP, tag="outsb")
            nc.vector.tensor_copy(out_sb[:], out_ps[:])
            nc.sync.dma_start(out=out[:], in_=out_sb[:])
    nc.compile()
    return nc


_NC_CACHE = None


def _get_nc():
    global _NC_CACHE
    if _NC_CACHE is None:
        _NC_CACHE = build_nc()
    return _NC_CACHE


def _make_in_maps(x, distance, Wcf_w, Wcf_b, Wdf_w, Wdf_b, Wfc_w):
    x = np.ascontiguousarray(np.asarray(x, np.float32))
    distance = np.ascontiguousarray(np.asarray(distance, np.float32))
    x_flat = x.reshape(B * N, D)
    dist_flat = distance.reshape(B * N, N * D)
    wcfT = np.asarray(Wcf_w, np.float32).T
    bcf = np.asarray(Wcf_b, np.float32)
    bdf = np.asarray(Wdf_b, np.float32)
    cstb = np.zeros((128, CB_TOT), bfloat16)
    cstb[:, CB_WDF:CB_WDF + H] = np.asarray(Wdf_w, np.float32).T.astype(bfloat16)
    cstb[:, CB_WFC:CB_WFC + D] = np.asarray(Wfc_w, np.float32).T.astype(bfloat16)
    cstb[:, CB_EYE:CB_EYE + ROWS] = np.eye(ROWS, dtype=np.float32).astype(bfloat16)
    in_maps = []
    for c in range(NCORES):
        sl = slice(c * ROWS, (c + 1) * ROWS)
        cstf = np.zeros((128, CF_TOT), np.float32)
        cstf[:, CF_XT:CF_XT + ROWS] = x_flat[sl].T
        cstf[:, CF_WCF:CF_WCF + H] = wcfT
        cstf[:, CF_BCF] = bcf
        cstf[:, CF_BDF] = bdf
        in_maps.append({
            "dist": np.ascontiguousarray(dist_flat[sl]),
            "cstf": cstf,
            "cstb": cstb,
        })
    return in_maps


def kernel(x, distance, Wcf_w, Wcf_b, Wdf_w, Wdf_b, Wfc_w):
    in_maps = _make_in_maps(x, distance, Wcf_w, Wcf_b, Wdf_w, Wdf_b, Wfc_w)
    nc = _get_nc()
    res = run_bass_kernel_spmd(nc, in_maps, list(range(NCORES))).results
    out = np.concatenate([res[c]["out"] for c in range(NCORES)], axis=0)
    return out.reshape(B, N, D)


# revision 4
# speedup vs baseline: 1.1085x; 1.0242x over previous
"""DTNN layer kernel for Trainium2 (8 NeuronCores).

Math: out[b,i,o] = sum_j sum_h Wfc[o,h] * hx[b,i,h] * hd[b,i,j,h]
with hx = x@Wcf.T + bcf, hd = dist@Wdf.T + bdf.
Since Wfc/Wdf are linear, the j-sum commutes:
    ds[b,i,d]  = sum_j dist[b,i,j,d]                  (memory-bound reduction)
    out[b,i,:] = ((x@Wcf.T + bcf) * (ds@Wdf.T + N*bdf)) @ Wfc.T
So the kernel streams `distance` once (134MB) and does a few 128x128 matmuls.

Sharding: flatten (B,N) -> 1024 i-rows, 128 rows per core; no cross-core comms.

Schedule (from NTFF traces of the 70us/69us predecessors):
- dist streams on the sync HWDGE queue as 17 tapered tiles
  [8,8,16,32x5,16,16,8,8,4,4,4,2,2]; two small tiles lead so folding starts
  ~15us (the DMA fabric ramps from ~300 to ~430 GB/s over the first ~25us,
  so early arrivals are slow), and the taper ends at 2j so the last fold is
  ~0.3us.  The two constant blocks ride 3rd/4th on the same queue -- on the
  scalar queue their 4KB packets serialize on one DMA engine and land at
  t=29us, stalling the whole in-order DVE program (the original 70us bug).
- Each tile folds to 128 columns by halving adds on DVE as it lands.  Level
  1 reads the f32 stream and writes bf16; levels >=2 are pure-bf16
  tensor_tensor adds which hit the DVE 2x_1p mode (2 elem/lane/cycle),
  cutting fold time ~35%.  All folds stay on DVE: a GpSimd-assist variant
  ran both engines ~2x slower from SBUF contention.
- PE transpose-accumulates each bf16 partial into a PSUM dsT bank (bf16
  matmul vs identity, ~0.4us single-pass; fp32 matmuls are dual-pass
  LOW_HIGH at ~1.6us).  This removes accumulator adds and the tail
  transpose entirely.
- Tail: last fold -> transpose-accum(stop) -> dsT copy (cast bf16) ->
  hd matmul -> sT mul -> out matmul (accumulates onto PSUM preloaded with
  the (hx*N*bdf)@WfcT bias term during the stream) -> copy -> DMA out.
- Numerics: bf16 fold partials + bf16 tail matmuls give rel err ~4e-3
  against the f32 reference (gate is 2e-2).
"""

import numpy as np
from ml_dtypes import bfloat16

import concourse.bass as bass
import concourse.bacc as bacc
import concourse.mybir as mybir
from concourse.tile import TileContext
from concourse.bass_utils import run_bass_kernel_spmd

B, N, D, H = 4, 256, 128, 128
NCORES = 8
ROWS = B * N // NCORES  # 128 i-rows per core
FP = mybir.dt.float32
BF = mybir.dt.bfloat16

# dist tile taper (j-columns per DMA); constants ride after the first two
SIZES = [8, 8, 16, 32, 32, 32, 32, 32, 16, 16, 8, 8, 4, 4, 4, 2, 2]
CST_AFTER = 2  # number of dist tiles DMA'd before the constant blocks
assert sum(SIZES) == N

# f32 constant block columns: [xT | wcfT | bcf_col | bdf_col]
CF_XT = 0
CF_WCF = 128
CF_BCF = 256
CF_BDF = 257
CF_TOT = 258
# bf16 constant block columns: [wdfT | wfcT | eye]
CB_WDF = 0
CB_WFC = 128
CB_EYE = 256
CB_TOT = 384


def build_nc():
    nc = bacc.Bacc("TRN2", target_bir_lowering=False)
    dist = nc.declare_dram_parameter("dist", [ROWS, N * D], FP, isOutput=False)
    cstf = nc.declare_dram_parameter("cstf", [128, CF_TOT], FP, isOutput=False)
    cstb = nc.declare_dram_parameter("cstb", [128, CB_TOT], BF, isOutput=False)
    out = nc.declare_dram_parameter("out", [ROWS, D], FP, isOutput=True)

    with TileContext(nc) as tc:
        with (
            tc.tile_pool(name="const", bufs=1) as cpool,
            tc.tile_pool(name="dist", bufs=1) as dpool,
            tc.tile_pool(name="work", bufs=1) as wpool,
            tc.tile_pool(name="psum", bufs=1, space="PSUM") as ppool,
        ):
            # DMA order on the sync queue: two small dist tiles (folding can
            # start ~15us), then the constants, then the rest of the stream.
            dtiles = []
            cstf_t = cstb_t = None
            off = 0
            for k, jn in enumerate(SIZES):
                if k == CST_AFTER:
                    cstf_t = cpool.tile([128, CF_TOT], FP, tag="cstf",
                                        name="cstf_t")
                    nc.sync.dma_start(out=cstf_t[:], in_=cstf[:])
                    cstb_t = cpool.tile([128, CB_TOT], BF, tag="cstb",
                                        name="cstb_t")
                    nc.sync.dma_start(out=cstb_t[:], in_=cstb[:])
                t = dpool.tile([ROWS, jn * D], FP, tag=f"dist{k}",
                               name=f"dist{k}_t")
                nc.sync.dma_start(out=t[:], in_=dist[:, off * D:(off + jn) * D])
                dtiles.append(t)
                off += jn

            xT_t = cstf_t[:, CF_XT:CF_XT + ROWS]
            wcf_t = cstf_t[:, CF_WCF:CF_WCF + H]
            bcf_col = cstf_t[:, CF_BCF:CF_BCF + 1]
            bdf_col = cstf_t[:, CF_BDF:CF_BDF + 1]
            wdf_b = cstb_t[:, CB_WDF:CB_WDF + H]
            wfc_b = cstb_t[:, CB_WFC:CB_WFC + D]
            eye_b = cstb_t[:, CB_EYE:CB_EYE + ROWS]

            # hx^T = (Wcf^T)^T @ x^T -> (H, ROWS) in PSUM (fp32)
            hx_ps = ppool.tile([H, ROWS], FP, tag="hx")
            nc.tensor.matmul(hx_ps[:], wcf_t, xT_t, start=True, stop=True)

            # bf16 halving scratch per tile (level-1 output and below)
            folds = [wpool.tile([ROWS, max(SIZES[k] // 2, 1) * D], BF,
                                tag=f"fold{k}", name=f"fold{k}_t")
                     for k in range(len(SIZES))]

            def emit_fold(k):
                # level 1: f32 tile halves -> bf16 scratch; levels >= 2:
                # bf16 in-place halving (DVE 2x_1p mode)
                t, fb, jn = dtiles[k], folds[k], SIZES[k]
                half = jn * D // 2
                nc.vector.tensor_add(fb[:, 0:half], t[:, 0:half],
                                     t[:, half:2 * half])
                while half > D:
                    h2 = half // 2
                    nc.vector.tensor_add(fb[:, 0:h2], fb[:, 0:h2],
                                         fb[:, h2:half])
                    half = h2

            emit_fold(0)

            # hx^T + bcf (f32), s0T = hxT * bdf * N (bf16, cast on write)
            hxT = wpool.tile([H, ROWS], FP, tag="hxT")
            nc.vector.tensor_scalar(hxT[:], hx_ps[:], bcf_col, None,
                                    mybir.AluOpType.add)
            s0T = wpool.tile([H, ROWS], BF, tag="s0T")
            nc.vector.tensor_scalar(s0T[:], hxT[:], bdf_col, float(N),
                                    mybir.AluOpType.mult,
                                    mybir.AluOpType.mult)

            for k in range(1, len(SIZES)):
                emit_fold(k)

            # Preload the bias term (hx * N*bdf) @ Wfc^T into the output
            # PSUM during the stream (bf16 matmul).
            out_ps = ppool.tile([ROWS, D], FP, tag="out")
            nc.tensor.matmul(out_ps[:], s0T[:], wfc_b, start=True, stop=False,
                             skip_group_check=True)

            # PE transpose-accumulate each bf16 partial into dsT PSUM:
            # dsT_ps += fold_k^T @ eye
            dsT_ps = ppool.tile([D, ROWS], FP, tag="dsT")
            for k in range(len(SIZES)):
                nc.tensor.matmul(dsT_ps[:], folds[k][:, 0:D], eye_b,
                                 start=(k == 0), stop=(k == len(SIZES) - 1),
                                 skip_group_check=True)

            dsT = wpool.tile([D, ROWS], BF, tag="dsTb")
            nc.vector.tensor_copy(dsT[:], dsT_ps[:])

            # hd^T (bias-free) = (Wdf^T)^T @ ds^T -> (H, ROWS), bf16 matmul
            hd_ps = ppool.tile([H, ROWS], FP, tag="hd")
            nc.tensor.matmul(hd_ps[:], wdf_b, dsT[:], start=True, stop=True,
                             skip_group_check=True)

            # s^T = hx^T * hd^T (bf16 out, cast on write)
            sT = wpool.tile([H, ROWS], BF, tag="sT")
            nc.vector.tensor_mul(sT[:], hd_ps[:], hxT[:])

            # out += sT^T @ Wfc^T, accumulating onto the preloaded bias term
            nc.tensor.matmul(out_ps[:], sT[:], wfc_b, start=False, stop=True,
                             skip_group_check=True)
            out_sb = wpool.tile([ROWS, D], FP, tag="outsb")
            nc.vector.tensor_copy(out_sb[:], out_ps[:])
            nc.sync.dma_start(out=out[:], in_=out_sb[:])
    nc.compile()
    return nc


_NC_CACHE = None


def _get_nc():
    global _NC_CACHE
    if _NC_CACHE is None:
        _NC_CACHE = build_nc()
    return _NC_CACHE


def _make_in_maps(x, distance, Wcf_w, Wcf_b, Wdf_w, Wdf_b, Wfc_w):
    x = np.ascontiguousarray(np.asarray(x, np.float32))
    distance = np.ascontiguousarray(np.asarray(distance, np.float32))
    x_flat = x.reshape(B * N, D)
    dist_flat = distance.reshape(B * N, N * D)
    wcfT = np.asarray(Wcf_w, np.float32).T
    bcf = np.asarray(Wcf_b, np.float32)
    bdf = np.asarray(Wdf_b, np.float32)
    cstb = np.zeros((128, CB_TOT), bfloat16)
    cstb[:, CB_WDF:CB_WDF + H] = np.asarray(Wdf_w, np.float32).T.astype(bfloat16)
    cstb[:, CB_WFC:CB_WFC + D] = np.asarray(Wfc_w, np.float32).T.astype(bfloat16)
    cstb[:, CB_EYE:CB_EYE + ROWS] = np.eye(ROWS, dtype=np.float32).astype(bfloat16)
    in_maps = []
    for c in range(NCORES):
        sl = slice(c * ROWS, (c + 1) * ROWS)
        cstf = np.zeros((128, CF_TOT), np.float32)
        cstf[:, CF_XT:CF_XT + ROWS] = x_flat[sl].T
        cstf[:, CF_WCF:CF_WCF + H] = wcfT
        cstf[:, CF_BCF] = bcf
        cstf[:, CF_BDF] = bdf
        in_maps.append({
            "dist": np.ascontiguousarray(dist_flat[sl]),
            "cstf": cstf,
            "cstb": cstb,
        })
    return in_maps


def kernel(x, distance, Wcf_w, Wcf_b, Wdf_w, Wdf_b, Wfc_w):
    in_maps = _make_in_maps(x, distance, Wcf_w, Wcf_b, Wdf_w, Wdf_b, Wfc_w)
    nc = _get_nc()
    res = run_bass_kernel_spmd(nc, in_maps, list(range(NCORES))).results
    out = np.concatenate([res[c]["out"] for c in range(NCORES)], axis=0)
    return out.reshape(B, N, D)


# revision 6
# speedup vs baseline: 1.2647x; 1.1409x over previous
"""DTNN layer kernel for Trainium2 (8 NeuronCores).

Math: out[b,i,o] = sum_j sum_h Wfc[o,h] * hx[b,i,h] * hd[b,i,j,h]
with hx = x@Wcf.T + bcf, hd = dist@Wdf.T + bdf.
Since Wfc/Wdf are linear, the j-sum commutes:
    ds[b,i,d]  = sum_j dist[b,i,j,d]                  (memory-bound reduction)
    out[b,i,:] = ((x@Wcf.T + bcf) * (ds@Wdf.T + N*bdf)) @ Wfc.T
So the kernel streams `distance` once (134MB) and does a few 128x128 matmuls.

Sharding: flatten (B,N) -> 1024 i-rows, 128 rows per core; no cross-core comms.

Schedule (from NTFF traces of the 70us/69us predecessors):
- dist streams on the sync HWDGE queue as 17 tapered tiles
  [8,8,16,32x5,16,16,8,8,4,4,4,2,2]; two small tiles lead so folding starts
  ~15us (the DMA fabric ramps from ~300 to ~430 GB/s over the first ~25us,
  so early arrivals are slow), and the taper ends at 2j so the last fold is
  ~0.3us.  The two constant blocks ride 3rd/4th on the same queue -- on the
  scalar queue their 4KB packets serialize on one DMA engine and land at
  t=29us, stalling the whole in-order DVE program (the original 70us bug).
- Each tile folds to 128 columns by halving adds on DVE as it lands.  Level
  1 reads the f32 stream and writes bf16; levels >=2 are pure-bf16
  tensor_tensor adds which hit the DVE 2x_1p mode (2 elem/lane/cycle),
  cutting fold time ~35%.  All folds stay on DVE: a GpSimd-assist variant
  ran both engines ~2x slower from SBUF contention.
- PE transpose-accumulates each bf16 partial into a PSUM dsT bank (bf16
  matmul vs identity, ~0.4us single-pass; fp32 matmuls are dual-pass
  LOW_HIGH at ~1.6us).  This removes accumulator adds and the tail
  transpose entirely.
- Tail: last fold -> transpose-accum(stop) -> dsT copy (cast bf16) ->
  hd matmul -> sT mul -> out matmul (accumulates onto PSUM preloaded with
  the (hx*N*bdf)@WfcT bias term during the stream) -> copy -> DMA out.
- Numerics: bf16 fold partials + bf16 tail matmuls give rel err ~4e-3
  against the f32 reference (gate is 2e-2).
"""

import numpy as np
from ml_dtypes import bfloat16

import concourse.bass as bass
import concourse.bacc as bacc
import concourse.mybir as mybir
from concourse.tile import TileContext
from concourse.bass_utils import run_bass_kernel_spmd

B, N, D, H = 4, 256, 128, 128
NCORES = 8
ROWS = B * N // NCORES  # 128 i-rows per core
FP = mybir.dt.float32
BF = mybir.dt.bfloat16

# dist tile taper (j-columns per DMA); constants ride after the first two
SIZES = [8, 16, 32, 32, 32, 32, 32, 16, 16, 16, 8, 8, 4, 2, 2]
CST_AFTER = 1  # number of dist tiles DMA'd before the constant blocks
assert sum(SIZES) == N

# f32 constant block columns: [xT | wcfT | bcf_col | bdf_col]
CF_XT = 0
CF_WCF = 128
CF_BCF = 256
CF_BDF = 257
CF_TOT = 258
# bf16 constant block columns: [wdfT | wfcT | eye]
CB_WDF = 0
CB_WFC = 128
CB_EYE = 256
CB_TOT = 384


def build_nc():
    nc = bacc.Bacc("TRN2", target_bir_lowering=False)
    dist = nc.declare_dram_parameter("dist", [ROWS, N * D], FP, isOutput=False)
    cstf = nc.declare_dram_parameter("cstf", [128, CF_TOT], FP, isOutput=False)
    cstb = nc.declare_dram_parameter("cstb", [128, CB_TOT], BF, isOutput=False)
    out = nc.declare_dram_parameter("out", [ROWS, D], FP, isOutput=True)

    with TileContext(nc) as tc:
        with (
            tc.tile_pool(name="const", bufs=1) as cpool,
            tc.tile_pool(name="dist", bufs=1) as dpool,
            tc.tile_pool(name="work", bufs=1) as wpool,
            tc.tile_pool(name="psum", bufs=1, space="PSUM") as ppool,
        ):
            # DMA order on the sync queue: two small dist tiles (folding can
            # start ~15us), then the constants, then the rest of the stream.
            dtiles = []
            cstf_t = cstb_t = None
            off = 0
            for k, jn in enumerate(SIZES):
                if k == CST_AFTER:
                    cstf_t = cpool.tile([128, CF_TOT], FP, tag="cstf",
                                        name="cstf_t")
                    nc.sync.dma_start(out=cstf_t[:], in_=cstf[:])
                    cstb_t = cpool.tile([128, CB_TOT], BF, tag="cstb",
                                        name="cstb_t")
                    nc.sync.dma_start(out=cstb_t[:], in_=cstb[:])
                t = dpool.tile([ROWS, jn * D], FP, tag=f"dist{k}",
                               name=f"dist{k}_t")
                nc.sync.dma_start(out=t[:], in_=dist[:, off * D:(off + jn) * D])
                dtiles.append(t)
                off += jn

            # Probe: one 64KB read on the scalar-engine HWDGE queue, result
            # unused.  The trace shows whether big descriptors stripe across
            # the 16 DMA engines on this queue (the 4KB-descriptor constant
            # block famously did not) -- informs a dual-queue stream variant.
            probe_t = cpool.tile([ROWS, D], FP, tag="probe", name="probe_t")
            nc.scalar.dma_start(out=probe_t[:], in_=dist[:, 0:D])

            xT_t = cstf_t[:, CF_XT:CF_XT + ROWS]
            wcf_t = cstf_t[:, CF_WCF:CF_WCF + H]
            bcf_col = cstf_t[:, CF_BCF:CF_BCF + 1]
            bdf_col = cstf_t[:, CF_BDF:CF_BDF + 1]
            wdf_b = cstb_t[:, CB_WDF:CB_WDF + H]
            wfc_b = cstb_t[:, CB_WFC:CB_WFC + D]
            eye_b = cstb_t[:, CB_EYE:CB_EYE + ROWS]

            # hx^T = (Wcf^T)^T @ x^T -> (H, ROWS) in PSUM (fp32)
            hx_ps = ppool.tile([H, ROWS], FP, tag="hx")
            nc.tensor.matmul(hx_ps[:], wcf_t, xT_t, start=True, stop=True)

            # bf16 halving scratch per tile (level-1 output and below)
            folds = [wpool.tile([ROWS, max(SIZES[k] // 2, 1) * D], BF,
                                tag=f"fold{k}", name=f"fold{k}_t")
                     for k in range(len(SIZES))]

            def emit_fold(k):
                # level 1: f32 tile halves -> bf16 scratch; levels >= 2:
                # bf16 in-place halving (DVE 2x_1p mode)
                t, fb, jn = dtiles[k], folds[k], SIZES[k]
                half = jn * D // 2
                nc.vector.tensor_add(fb[:, 0:half], t[:, 0:half],
                                     t[:, half:2 * half])
                while half > D:
                    h2 = half // 2
                    nc.vector.tensor_add(fb[:, 0:h2], fb[:, 0:h2],
                                         fb[:, h2:half])
                    half = h2

            emit_fold(0)

            # hx^T + bcf (f32), s0T = hxT * bdf * N (bf16, cast on write)
            hxT = wpool.tile([H, ROWS], FP, tag="hxT")
            nc.vector.tensor_scalar(hxT[:], hx_ps[:], bcf_col, None,
                                    mybir.AluOpType.add)
            s0T = wpool.tile([H, ROWS], BF, tag="s0T")
            nc.vector.tensor_scalar(s0T[:], hxT[:], bdf_col, float(N),
                                    mybir.AluOpType.mult,
                                    mybir.AluOpType.mult)

            for k in range(1, len(SIZES)):
                emit_fold(k)

            # Preload the bias term (hx * N*bdf) @ Wfc^T into the output
            # PSUM during the stream (bf16 matmul).
            out_ps = ppool.tile([ROWS, D], FP, tag="out")
            nc.tensor.matmul(out_ps[:], s0T[:], wfc_b, start=True, stop=False,
                             skip_group_check=True)

            # PE transpose-accumulate each bf16 partial into dsT PSUM:
            # dsT_ps += fold_k^T @ eye
            dsT_ps = ppool.tile([D, ROWS], FP, tag="dsT")
            for k in range(len(SIZES)):
                nc.tensor.matmul(dsT_ps[:], folds[k][:, 0:D], eye_b,
                                 start=(k == 0), stop=(k == len(SIZES) - 1),
                                 skip_group_check=True)

            dsT = wpool.tile([D, ROWS], BF, tag="dsTb")
            nc.vector.tensor_copy(dsT[:], dsT_ps[:])

            # hd^T (bias-free) = (Wdf^T)^T @ ds^T -> (H, ROWS), bf16 matmul
            hd_ps = ppool.tile([H, ROWS], FP, tag="hd")
            nc.tensor.matmul(hd_ps[:], wdf_b, dsT[:], start=True, stop=True,
                             skip_group_check=True)

            # s^T = hx^T * hd^T (bf16 out, cast on write)
            sT = wpool.tile([H, ROWS], BF, tag="sT")
            nc.vector.tensor_mul(sT[:], hd_ps[:], hxT[:])

            # out += sT^T @ Wfc^T, accumulating onto the preloaded bias term
            nc.tensor.matmul(out_ps[:], sT[:], wfc_b, start=False, stop=True,
                             skip_group_check=True)
            out_sb = wpool.tile([ROWS, D], FP, tag="outsb")
            nc.vector.tensor_copy(out_sb[:], out_ps[:])
            nc.sync.dma_start(out=out[:], in_=out_sb[:])
    nc.compile()
    return nc


_NC_CACHE = None


def _get_nc():
    global _NC_CACHE
    if _NC_CACHE is None:
        _NC_CACHE = build_nc()
    return _NC_CACHE


def _make_in_maps(x, distance, Wcf_w, Wcf_b, Wdf_w, Wdf_b, Wfc_w):
    x = np.ascontiguousarray(np.asarray(x, np.float32))
    distance = np.ascontiguousarray(np.asarray(distance, np.float32))
    x_flat = x.reshape(B * N, D)
    dist_flat = distance.reshape(B * N, N * D)
    wcfT = np.asarray(Wcf_w, np.float32).T
    bcf = np.asarray(Wcf_b, np.float32)
    bdf = np.asarray(Wdf_b, np.float32)
    cstb = np.zeros((128, CB_TOT), bfloat16)
    cstb[:, CB_WDF:CB_WDF + H] = np.asarray(Wdf_w, np.float32).T.astype(bfloat16)
    cstb[:, CB_WFC:CB_WFC + D] = np.asarray(Wfc_w, np.float32).T.astype(bfloat16)
    cstb[:, CB_EYE:CB_EYE + ROWS] = np.eye(ROWS, dtype=np.float32).astype(bfloat16)
    in_maps = []
    for c in range(NCORES):
        sl = slice(c * ROWS, (c + 1) * ROWS)
        cstf = np.zeros((128, CF_TOT), np.float32)
        cstf[:, CF_XT:CF_XT + ROWS] = x_flat[sl].T
        cstf[:, CF_WCF:CF_WCF + H] = wcfT
        cstf[:, CF_BCF] = bcf
        cstf[:, CF_BDF] = bdf
        in_maps.append({
            "dist": np.ascontiguousarray(dist_flat[sl]),
            "cstf": cstf,
            "cstb": cstb,
        })
    return in_maps


def kernel(x, distance, Wcf_w, Wcf_b, Wdf_w, Wdf_b, Wfc_w):
    in_maps = _make_in_maps(x, distance, Wcf_w, Wcf_b, Wdf_w, Wdf_b, Wfc_w)
    nc = _get_nc()
    res = run_bass_kernel_spmd(nc, in_maps, list(range(NCORES))).results
    out = np.concatenate([res[c]["out"] for c in range(NCORES)], axis=0)
    return out.reshape(B, N, D)
